# revision 1
# baseline (speedup 1.0000x reference)
"""Trainium2 Bass kernel for AtomicMNISTClassifier (3-layer MLP + log_softmax).

Data-parallel across 8 NeuronCores: batch 32768 -> 4096 rows/core, weights
replicated. Host pre-processing casts x to bf16 and lays it out feature-major
(x.T), and pre-transposes/casts the weights, so the device only ever issues
large plain DMAs.

Per-core pipeline (8 batch chunks of 512 rows, software-pipelined with the
L1 matmuls of later chunks running 2 chunks ahead on the PE):
  x.T arrives in `xq` batch slices, one big DMA each; only the 784 real
     feature rows are transferred (zero-pad rows are skipped; the last
     contraction chunk uses K=16).
  L1: c-outer over groups of 4 chunks — each of the 7 w1 chunks is loaded
     once per group and streamed against all 4 chunks' PSUM accumulators
     back-to-back (dense PE bursts, 4x fewer weight loads);
      ScalarE epilogue relu(psum + b1) writes bf16 SBUF directly (PSUM-read
      activations may narrow to bf16; SBUF->SBUF copy-casts hang instead —
      the epi_bf16=False path keeps the strided-truncation workaround).
  L2: 2 matmuls (weights stationary) + ScalarE relu epilogues -> h2 bf16.
  L3 (acts stationary): per 128-row subtile, lhsT = h2 slice, rhs = w3.T,
      so logits land batch-major [128,10] in PSUM and nothing downstream
      blocks the PE stream; b3 is added on VectorE from a host-replicated
      [128,10] tile.
  log_softmax: per-subtile max-reduce/subtract on VectorE; one batched Exp
      and one batched Ln per chunk on ScalarE; final subtract on VectorE.
  Outputs stage in SBUF; one DMA per 1024 rows.

All ScalarE functions used share one ACT table set
(`natural_log_exp_and_others`), so no table reloads occur. Known-broken
constructs on this stack, found empirically and avoided here: any engine op
copy-casting f32->bf16 SBUF->SBUF (hangs), dma_start_transpose on the Activation HWDGE
queue (corrupts), K=1 matmuls (device crash), tensor_tensor_reduce
(INTERNAL error), float32r matmuls (compile failure).
"""

import sys

for _p in ("/opt/trn_rl_repo",):
    if _p not in sys.path:
        sys.path.insert(0, _p)

import ml_dtypes
import numpy as np

import concourse.bass as bass
import concourse.tile as tile
from concourse import bacc, mybir
from concourse.bass import ts
from concourse.bass_utils import run_bass_kernel_spmd
from concourse.masks import make_identity

# All activation funcs we use live in one ACT table set; restricting the
# table list keeps bacc from inserting per-function table reloads (~2.7us
# each on hardware).
import concourse.bacc as _bacc_mod
import concourse.hw_specs as _hw_specs

_orig_get_tables = _hw_specs.get_activation_tables


def _only_nl_exp(arch):
    # Keep every entry (act_func_set_id is the index into the full list)
    # but empty all sets except the one that covers Relu/Exp/Ln/Identity/
    # Copy, so the chooser can only pick that one.
    t = _orig_get_tables(arch)
    return {
        k: (v if k == "natural_log_exp_and_others" else set())
        for k, v in t.items()
    }


_bacc_mod.get_activation_tables = _only_nl_exp

N_CORES = 8
B_FULL = 32768
B_SH = B_FULL // N_CORES  # 4096
F_IN = 784
F_PAD = 896  # 7 * 128
H1 = 128
H2 = 256
NCLS = 10
CHUNK_B = 512
N_CHUNKS = B_SH // CHUNK_B  # 8
SUB = CHUNK_B // 128  # 4
NFC = F_PAD // 128  # 7

F32 = mybir.dt.float32
BF16 = mybir.dt.bfloat16
AFT = mybir.ActivationFunctionType
ALU = mybir.AluOpType


def _trunc_bf16_view(ap_f32):
    """bf16 high-half (truncation) view of an f32 AP, same shape."""
    v = ap_f32.bitcast(BF16)
    if len(v.shape) == 2:
        return v.rearrange("p (n two) -> p n two", two=2)[:, :, 1]
    return v.rearrange("p q (n two) -> p q n two", two=2)[:, :, :, 1]


def build(reps=None, skew=2, h_bufs=4, mm_bufs=6, l3_bufs=2, h1_on_dve=True, h2_on_dve=False, trunc_on='dve', trunc_split=False, xq=8, body_reps=1, l3_delay=False, epi_bf16=True, epi_h1_dve=False, epi_h2_split=False, l1_group=4, out_per_chunk=False, sm_bufs=8):
    """Build the kernel graph. With reps=N, the whole per-core pipeline is
    wrapped in a runtime For_i loop that executes it N times — used only for
    benchmarking (slope of wall time vs reps = device time per pipeline).
    The remaining knobs are pipeline-tuning parameters."""
    nc = bacc.Bacc(
        "TRN2", target_bir_lowering=False, debug=False, num_devices=N_CORES
    )
    x_d = nc.dram_tensor("xt", [F_PAD, B_SH], BF16, kind="ExternalInput").ap()
    w1t_d = nc.dram_tensor("w1t", [F_PAD, H1], BF16, kind="ExternalInput").ap()
    w2t_d = nc.dram_tensor("w2t", [H1, H2], BF16, kind="ExternalInput").ap()
    w3t_d = nc.dram_tensor("w3t", [H2, NCLS], BF16, kind="ExternalInput").ap()
    b1_d = nc.dram_tensor("b1", [H1], F32, kind="ExternalInput").ap()
    b2_d = nc.dram_tensor("b2", [H2], F32, kind="ExternalInput").ap()
    b3bc_d = nc.dram_tensor("b3bc", [128, NCLS], F32, kind="ExternalInput").ap()
    out_d = nc.dram_tensor("out", [B_SH, NCLS], F32, kind="ExternalOutput").ap()

    with tile.TileContext(nc) as tc:
        with (
            tc.tile_pool(name="consts", bufs=1) as consts,
            tc.tile_pool(name="xt", bufs=1) as xt_pool,
            tc.tile_pool(name="h", bufs=h_bufs) as h_pool,
            tc.tile_pool(name="sm", bufs=sm_bufs) as sm_pool,
            tc.tile_pool(name="ob", bufs=1) as ob_pool,
            tc.tile_pool(name="mm_psum", bufs=mm_bufs, space="PSUM") as mm_psum,
            tc.tile_pool(name="l3_psum", bufs=l3_bufs, space="PSUM") as l3_psum,
        ):
            # ---- one-time: weights and biases

            w1t = consts.tile([128, NFC, H1], BF16)
            nc.sync.dma_start(
                w1t[:], w1t_d[:].rearrange("(c k) m -> k c m", k=128)
            )
            w2t = consts.tile([128, 2, 128], BF16)
            nc.sync.dma_start(
                w2t[:], w2t_d[:].rearrange("k (c m) -> k c m", m=128)
            )
            w3t = consts.tile([128, 2, NCLS], BF16)
            nc.sync.dma_start(
                w3t[:], w3t_d[:].rearrange("(c k) m -> k c m", k=128)
            )
            b1c = consts.tile([H1, 1], F32)
            nc.sync.dma_start(b1c[:], b1_d[:])
            b2c0 = consts.tile([128, 1], F32)
            b2c1 = consts.tile([128, 1], F32)
            b2c = [b2c0, b2c1]
            for h in range(2):
                nc.sync.dma_start(b2c[h][:], b2_d[ts(h, 128)])
            b3bc = consts.tile([128, NCLS], F32)
            nc.sync.dma_start(b3bc[:], b3bc_d[:])

            # ---- main pipeline over batch chunks of 512 rows
            import contextlib

            loop_ctx = (
                tc.For_i(0, reps, 1) if reps else contextlib.nullcontext()
            )
            with loop_ctx:
                for _ in range(body_reps):
                    _emit_pipeline(
                        nc, tc, x_d, out_d, w1t, w2t, w3t, b1c, b2c, b3bc,
                        xt_pool, h_pool, sm_pool, ob_pool, mm_psum, l3_psum,
                        skew=skew, h1_on_dve=h1_on_dve, h2_on_dve=h2_on_dve,
                        trunc_on=trunc_on, trunc_split=trunc_split, xq=xq,
                        l3_delay=l3_delay, epi_bf16=epi_bf16,
                        epi_h1_dve=epi_h1_dve, epi_h2_split=epi_h2_split,
                        l1_group=l1_group, out_per_chunk=out_per_chunk,
                    )

    nc.compile()
    return nc


def _trunc_eng(nc, which):
    return {
        "dve": nc.vector.tensor_copy,
        "gpsimd": nc.gpsimd.tensor_copy,
        "act": lambda out, in_: nc.scalar.activation(out, in_, AFT.Copy),
    }[which]


def _emit_pipeline(
    nc, tc, x_d, out_d, w1t, w2t, w3t, b1c, b2c, b3bc,
    xt_pool, h_pool, sm_pool, ob_pool, mm_psum, l3_psum,
    skew=2, h1_on_dve=True, h2_on_dve=False, trunc_on="dve",
    trunc_split=False, xq=8, l3_delay=False, epi_bf16=True,
    epi_h1_dve=False, epi_h2_split=False, l1_group=0,
    out_per_chunk=False,
):
    # x arrives already feature-major (host-transposed); four big plain
    # DMAs, one per batch quarter, so compute on quarter q starts as soon
    # as its own transfer lands.
    QB = B_SH // xq
    xtq = []
    for q in range(xq):
        xt_q = xt_pool.tile([128, NFC, QB], BF16, tag=f"xtq{q}")
        xtq.append(xt_q)
        nc.sync.dma_start(
            xt_q[:, 0:6, :],
            x_d[0:768, ts(q, QB)].rearrange("(c k) b -> k c b", k=128),
        )
        nc.sync.dma_start(xt_q[0:16, 6, :], x_d[ts(48, 16), ts(q, QB)])

    oball = ob_pool.tile([128, N_CHUNKS, SUB, NCLS], F32)

    def emit_l1(chunk):
        # L1: h1 = relu(x @ w1.T + b1)   [128 fo, 512 b]
        l1p = mm_psum.tile([128, CHUNK_B], F32, tag="mm")
        for c in range(NFC):
            kk = 128 if c < 6 else 16
            nc.tensor.matmul(
                l1p[:],
                w1t[0:kk, c, :],
                xtq[chunk * CHUNK_B // QB][0:kk, c, ts(chunk % (QB // CHUNK_B), CHUNK_B)],
                start=(c == 0),
                stop=(c == NFC - 1),
            )
        return l1p

    def emit_rest(chunk, l1p):
        if epi_bf16:
            # PSUM-read epilogues may write bf16 SBUF directly (HW-proven;
            # only SBUF->SBUF copy-casts hang on this stack). h1 goes on
            # VectorE as a fused add+max so ScalarE and VectorE alternate
            # along the per-chunk chain.
            h1b = h_pool.tile([128, CHUNK_B], BF16, tag="h1b")
            if epi_h1_dve:
                nc.vector.tensor_scalar(
                    h1b[:], l1p[:], b1c[:], 0.0, ALU.add, ALU.max
                )
            else:
                nc.scalar.activation(h1b[:], l1p[:], AFT.Relu, bias=b1c[:])
            h2b = h_pool.tile([128, 2, CHUNK_B], BF16, tag="h2b")
            for h in range(2):
                l2p = mm_psum.tile([128, CHUNK_B], F32, tag="mm")
                nc.tensor.matmul(l2p[:], w2t[:, h, :], h1b[:])
                if epi_h2_split and h == 0:
                    nc.vector.tensor_scalar(
                        h2b[:, h, :], l2p[:], b2c[h][:], 0.0,
                        ALU.add, ALU.max,
                    )
                else:
                    nc.scalar.activation(
                        h2b[:, h, :], l2p[:], AFT.Relu, bias=b2c[h][:]
                    )
            return h2b
        h1f = h_pool.tile([128, CHUNK_B], F32, tag="h1f")
        if h1_on_dve:
            nc.vector.tensor_scalar(
                h1f[:], l1p[:], b1c[:], 0.0, ALU.add, ALU.max
            )
        else:
            nc.scalar.activation(h1f[:], l1p[:], AFT.Relu, bias=b1c[:])
        h1b = h_pool.tile([128, CHUNK_B], BF16, tag="h1b")
        if trunc_split:
            hv = _trunc_bf16_view(h1f[:])
            half = CHUNK_B // 2
            nc.vector.tensor_copy(h1b[:, 0:half], hv[:, 0:half])
            nc.scalar.activation(
                h1b[:, half:CHUNK_B], hv[:, half:CHUNK_B], AFT.Copy
            )
        else:
            _trunc_eng(nc, trunc_on)(h1b[:], _trunc_bf16_view(h1f[:]))

        # L2: h2 = relu(h1 @ w2.T + b2)  [256 fo, 512 b] in halves
        h2f = h_pool.tile([128, 2, CHUNK_B], F32, tag="h2f")
        h2b = h_pool.tile([128, 2, CHUNK_B], BF16, tag="h2b")
        for h in range(2):
            l2p = mm_psum.tile([128, CHUNK_B], F32, tag="mm")
            nc.tensor.matmul(l2p[:], w2t[:, h, :], h1b[:])
            if h2_on_dve:
                nc.vector.tensor_scalar(
                    h2f[:, h, :], l2p[:], b2c[h][:], 0.0, ALU.add, ALU.max
                )
            else:
                nc.scalar.activation(
                    h2f[:, h, :], l2p[:], AFT.Relu, bias=b2c[h][:]
                )
            if trunc_split:
                hv = _trunc_bf16_view(h2f[:, h, :])
                half = CHUNK_B // 2
                nc.vector.tensor_copy(h2b[:, h, 0:half], hv[:, 0:half])
                nc.scalar.activation(
                    h2b[:, h, half:CHUNK_B], hv[:, half:CHUNK_B], AFT.Copy
                )
            else:
                _trunc_eng(nc, trunc_on)(
                    h2b[:, h, :], _trunc_bf16_view(h2f[:, h, :])
                )

        return h2b

    def emit_l3sm(chunk, h2b):
        # L3 acts-stationary: per 128-row subtile, logits[128b, 10] land
        # batch-major in PSUM, so no logit transpose is needed and nothing
        # downstream blocks the PE stream. b3 is added on VectorE from a
        # host-replicated tile.
        tcat = sm_pool.tile([128, SUB, NCLS], F32, tag="tcat")
        for s in range(SUB):
            l3p = l3_psum.tile([128, NCLS], F32, tag="l3")
            for c in range(2):
                nc.tensor.matmul(
                    l3p[:],
                    h2b[:, c, ts(s, 128)],
                    w3t[:, c, :],
                    start=(c == 0),
                    stop=(c == 1),
                )
            # logits = l3p + b3 (broadcast tile), then max-subtract
            tls = sm_pool.tile([128, NCLS], F32, tag="tls")
            nc.vector.tensor_tensor(tls[:], l3p[:], b3bc[:], ALU.add)
            nmax = sm_pool.tile([128, 1], F32, tag="nmax")
            nc.vector.tensor_reduce(
                nmax[:], tls[:], axis=mybir.AxisListType.X,
                op=ALU.max, negate=True,
            )
            nc.vector.tensor_scalar(
                tcat[:, s, :], tls[:], nmax[:], None, ALU.add
            )
        ecat = sm_pool.tile([128, SUB, NCLS], F32, tag="ecat")
        nc.scalar.activation(ecat[:], tcat[:], AFT.Exp)
        ss4 = sm_pool.tile([128, SUB], F32, tag="ss4")
        nc.vector.tensor_reduce(
            ss4[:], ecat[:], axis=mybir.AxisListType.X, op=ALU.add
        )
        ln4 = sm_pool.tile([128, SUB], F32, tag="ln4")
        nc.scalar.activation(ln4[:], ss4[:], AFT.Ln)
        for s in range(SUB):
            nc.vector.tensor_scalar(
                oball[:, chunk, s, :], tcat[:, s, :], ln4[:, s : s + 1],
                None, ALU.subtract,
            )

    def flush_out_grouped(done_chunk):
        if out_per_chunk:
            # one small flush per chunk, alternating normal-mode HWDGE
            # queues, so the final flush covers only 512 rows
            eng = nc.sync if done_chunk % 2 == 0 else nc.scalar
            odst = out_d[ts(done_chunk, CHUNK_B), :].rearrange(
                "(s p) f -> p s f", p=128
            )
            eng.dma_start(odst, oball[:, done_chunk, :, :])
            return
        if done_chunk % 2 == 1:
            pair = done_chunk // 2
            odst = out_d[ts(pair, 2 * CHUNK_B), :].rearrange(
                "(k s p) f -> p k s f", p=128, s=SUB
            )
            nc.sync.dma_start(odst, oball[:, ts(pair, 2), :, :])

    def flush_out(done_chunk):
        # flush a finished pair of chunks while later chunks compute
        if done_chunk % 2 == 1:
            pair = done_chunk // 2
            odst = out_d[ts(pair, 2 * CHUNK_B), :].rearrange(
                "(k s p) f -> p k s f", p=128, s=SUB
            )
            nc.sync.dma_start(odst, oball[:, ts(pair, 2), :, :])

    def emit_l1_group(g):
        # c-outer over a group of chunks: each w1t chunk is loaded once per
        # group and streamed against all group members back-to-back, so the
        # PE gets a dense burst and weight loads amortize.
        l1ps = []
        for j in range(l1_group):
            l1pj = mm_psum.tile([128, CHUNK_B], F32, tag="mm")
            l1ps.append(l1pj)
        for c in range(NFC):
            kk = 128 if c < 6 else 16
            for j, l1pj in enumerate(l1ps):
                chunk = g * l1_group + j
                nc.tensor.matmul(
                    l1pj[:],
                    w1t[0:kk, c, :],
                    xtq[chunk * CHUNK_B // QB][
                        0:kk, c, ts(chunk % (QB // CHUNK_B), CHUNK_B)
                    ],
                    start=(c == 0),
                    stop=(c == NFC - 1),
                )
        return l1ps

    if l1_group:
        n_groups = N_CHUNKS // l1_group
        gpend = [emit_l1_group(0)]
        for g in range(n_groups):
            if g + 1 < n_groups:
                gpend.append(emit_l1_group(g + 1))
            l1ps = gpend.pop(0)
            for j in range(l1_group):
                chunk = g * l1_group + j
                h2b_cur = emit_rest(chunk, l1ps[j])
                emit_l3sm(chunk, h2b_cur)
                flush_out_grouped(chunk)
        return

    # software-pipelined emission: L1 of later chunks is queued on the PE
    # before chunk i's L2 (skew), and with l3_delay each chunk's L3+softmax
    # is emitted one chunk later so its h2b input is long since ready when
    # the PE reaches the L3 weight loads.
    pend = [emit_l1(i) for i in range(skew)]
    l3_pend = None  # (chunk, h2b) awaiting L3+softmax emission
    for chunk in range(N_CHUNKS):
        if chunk + skew < N_CHUNKS:
            pend.append(emit_l1(chunk + skew))
        h2b_cur = emit_rest(chunk, pend.pop(0))
        if l3_delay:
            if l3_pend is not None:
                emit_l3sm(*l3_pend)
                flush_out(l3_pend[0])
            l3_pend = (chunk, h2b_cur)
        else:
            emit_l3sm(chunk, h2b_cur)
            flush_out(chunk)
    if l3_pend is not None:
        emit_l3sm(*l3_pend)
        flush_out(l3_pend[0])


_NC_CACHE = {}


def _get_nc():
    if "nc" not in _NC_CACHE:
        _NC_CACHE["nc"] = build()
    return _NC_CACHE["nc"]


def _prep_host(x, w1, b1, w2, b2, w3, b3):
    xf = np.asarray(x, dtype=np.float32).reshape(B_FULL, F_IN)
    xb = np.zeros((F_PAD, B_FULL), dtype=ml_dtypes.bfloat16)
    xb[:F_IN, :] = xf.T.astype(ml_dtypes.bfloat16)
    w1tb = np.zeros((F_PAD, H1), dtype=ml_dtypes.bfloat16)
    w1tb[:F_IN, :] = np.asarray(w1, np.float32).T.astype(ml_dtypes.bfloat16)
    w2tb = np.ascontiguousarray(
        np.asarray(w2, np.float32).T.astype(ml_dtypes.bfloat16)
    )
    w3tb = np.ascontiguousarray(
        np.asarray(w3, np.float32).T.astype(ml_dtypes.bfloat16)
    )
    reps = {
        "w1t": w1tb,
        "w2t": w2tb,
        "w3t": w3tb,
        "b1": np.ascontiguousarray(np.asarray(b1, np.float32)),
        "b2": np.ascontiguousarray(np.asarray(b2, np.float32)),
        "b3bc": np.ascontiguousarray(
            np.tile(np.asarray(b3, np.float32).reshape(1, NCLS), (128, 1))
        ),
    }
    return xb, reps


def kernel(x, w1, b1, w2, b2, w3, b3, _trace=False, **run_kwargs):
    nc = _get_nc()
    xb, reps = _prep_host(x, w1, b1, w2, b2, w3, b3)
    in_maps = [
        {
            "xt": np.ascontiguousarray(xb[:, i * B_SH : (i + 1) * B_SH]),
            **reps,
        }
        for i in range(N_CORES)
    ]
    res = run_bass_kernel_spmd(
        nc, in_maps, core_ids=list(range(N_CORES)), trace=_trace, **run_kwargs
    )
    out = np.concatenate(
        [res.results[i]["out"] for i in range(N_CORES)], axis=0
    )
    if _trace:
        return out, res
    return out


if __name__ == "__main__":
    rng = np.random.default_rng(0)
    ins = {
        "x": rng.standard_normal((B_FULL, 1, 28, 28), dtype=np.float32),
        "w1": rng.standard_normal((H1, F_IN), dtype=np.float32),
        "b1": rng.standard_normal((H1,), dtype=np.float32),
        "w2": rng.standard_normal((H2, H1), dtype=np.float32),
        "b2": rng.standard_normal((H2,), dtype=np.float32),
        "w3": rng.standard_normal((NCLS, H2), dtype=np.float32),
        "b3": rng.standard_normal((NCLS,), dtype=np.float32),
    }
    out = kernel(**ins)
    print("out:", out.shape, out.dtype)

